# revision 6
# baseline (speedup 1.0000x reference)
"""Causal multi-head attention (B=2, S=2048, D=1024, H=16) on 8 NeuronCores.

Sharding: batch x head-group (2 x 4). Core c handles batch b = c // 4 and
head group g = c % 4 (4 heads = 256 model dims). Each core computes a full
[S, D] partial of the output projection for its batch; the host sums the 4
group partials per batch and adds the bias.

v2 (cost model: matmul time = out free-dim cols x pe_cycle, so the PV matmul
is flipped to put q on the output partitions):
  Phase 1 (per half,m): KT/QT k-outer accumulation, N=512 matmuls.
  Attention per block (j, pair):
    A: for kb ascending: ST = K_h @ Q_h^T [128 k, 512 q] x2 heads;
       P = exp(0.125*ST) (ACT -> SBUF bf16); diag-block mask on DVE.
       All 16 P tiles of the block stay resident.
    B (drained as PE fill into the next block's A): per (qq, hh) one PSUM
       accumulation group over kb: ctx[q, hd|den] += P_kb^T @ [V_kb|1],
       lhsT = P tile [128k,128q], rhs [128k,65] -> N=65, den rides as col 64.
       Then recip (DVE), normalize (tensor_scalar), and a DMA transpose
       [128q,128hd] -> ctxT [128hd,128q].
  out_partial = ctxT^T @ Wo_g [2048, 1024] -> bf16 -> DRAM.
PSUM banks: sc0(2) sc1(2) ctxA ctxB vo vo2.
"""

from collections import deque

import numpy as np
import ml_dtypes

B, S, D, H = 2, 2048, 1024, 16
HD = 64
G = 4            # head groups = tensor-parallel degree per batch
GD = D // G      # 256 dims per group
N_CORES = 8
BF16 = ml_dtypes.bfloat16

USE_DMA_TRANSPOSE = True

_CACHE = {}


def _MARK(label):  # analysis hook, no-op in production
    pass


def _build_nc():
    import concourse.mybir as mybir
    from concourse import bacc
    from concourse.tile import TileContext

    dt = mybir.dt
    f32 = dt.float32
    bf16 = dt.bfloat16
    AF = mybir.ActivationFunctionType

    nc = bacc.Bacc("TRN2")

    xT = nc.dram_tensor("xT", [D, S], bf16, kind="ExternalInput")
    wq = nc.dram_tensor("wq", [D, GD], bf16, kind="ExternalInput")
    wk = nc.dram_tensor("wk", [D, GD], bf16, kind="ExternalInput")
    wv = nc.dram_tensor("wv", [D, GD], bf16, kind="ExternalInput")
    wo = nc.dram_tensor("wo", [GD, D], bf16, kind="ExternalInput")
    masks = nc.dram_tensor("masks", [128, 1152], bf16, kind="ExternalInput")
    outp = nc.dram_tensor("out", [S, D], bf16, kind="ExternalOutput")

    KC = D // 128   # 8 contraction chunks of 128
    NQ = S // 512   # 4 q blocks
    VW = HD + 1     # 65: V columns + the denominator ones-column

    with TileContext(nc) as tc:
        with tc.tile_pool(name="const", bufs=1) as cpool:
            HS = S // 2

            def load_xt(k, hf):
                t = cpool.tile([128, HS], bf16, tag=f"xT{k}_{hf}",
                               name=f"xt{k}_{hf}")
                nc.sync.dma_start(
                    out=t, in_=xT[128 * k:128 * (k + 1), HS * hf:HS * (hf + 1)])
                return t

            def load_w(nm, dram):
                t = cpool.tile([128, KC * GD], bf16, tag=nm, name=nm + "_t")
                nc.sync.dma_start(
                    out=t.rearrange("p (c g) -> p c g", c=KC),
                    in_=dram.rearrange("(c p) g -> p c g", p=128))
                return [t[:, k * GD:(k + 1) * GD] for k in range(KC)]

            # c0 weight slivers + first xT half-chunk first so PE starts
            # ASAP; the first chunk is split so the k=0 matmul can start
            # after only 512 columns have landed
            wkc0 = cpool.tile([128, GD], bf16, tag="wkc0")
            nc.sync.dma_start(out=wkc0, in_=wk[0:128, :])
            xt00 = cpool.tile([128, HS], bf16, tag="xT0_0", name="xt0_0")
            nc.sync.dma_start(out=xt00[:, 0:512], in_=xT[0:128, 0:512])
            nc.sync.dma_start(out=xt00[:, 512:HS], in_=xT[0:128, 512:HS])
            wqc0 = cpool.tile([128, GD], bf16, tag="wqc0")
            nc.sync.dma_start(out=wqc0, in_=wq[0:128, :])
            xt_a = [xt00]
            wk_rest = cpool.tile([128, (KC - 1) * GD], bf16, tag="wk")
            nc.sync.dma_start(
                out=wk_rest[:, 0:2 * GD].rearrange("p (c g) -> p c g", c=2),
                in_=wk[128:384, :].rearrange("(c p) g -> p c g", p=128))
            xt_a.append(load_xt(1, 0))
            nc.sync.dma_start(
                out=wk_rest[:, 2 * GD:].rearrange("p (c g) -> p c g", c=KC - 3),
                in_=wk[384:, :].rearrange("(c p) g -> p c g", p=128))
            wq_rest = cpool.tile([128, (KC - 1) * GD], bf16, tag="wq")
            nc.sync.dma_start(
                out=wq_rest[:, 0:2 * GD].rearrange("p (c g) -> p c g", c=2),
                in_=wq[128:384, :].rearrange("(c p) g -> p c g", p=128))
            nc.sync.dma_start(
                out=wq_rest[:, 2 * GD:].rearrange("p (c g) -> p c g", c=KC - 3),
                in_=wq[384:, :].rearrange("(c p) g -> p c g", p=128))
            wq_sb = [wqc0] + [wq_rest[:, k * GD:(k + 1) * GD] for k in range(KC - 1)]
            wk_sb = [wkc0] + [wk_rest[:, k * GD:(k + 1) * GD] for k in range(KC - 1)]
            for k in range(2, KC):
                xt_a.append(load_xt(k, 0))
            wv_sb = load_w("wv", wv)
            mask_sb = cpool.tile([128, 1152], bf16, tag="masks")
            nc.sync.dma_start(out=mask_sb, in_=masks[:, :])
            xt_b = [load_xt(k, 1) for k in range(KC)]
            wo_sb = cpool.tile([128, 2 * D], bf16, tag="wo")
            nc.sync.dma_start(
                out=wo_sb.rearrange("p (c o) -> p c o", c=2),
                in_=wo.rearrange("(c p) o -> p c o", p=128))
            xt_half = (xt_a, xt_b)
            ident_sb = mask_sb[:, 1024:1152]

            # persistent per-core intermediates
            qT_sb = cpool.tile([128, 2 * S], bf16, tag="qT")    # pair m at cols m*S
            kT_sb = cpool.tile([128, 2 * S], bf16, tag="kT")
            # V row tile r at cols (r%4)*4*VW, head h at +h*VW; col 64 = ones
            v_sb = []
            for jj in range(NQ):
                t = cpool.tile([128, 4 * 4 * VW], bf16, tag=f"v{jj}",
                               name=f"v{jj}")
                nc.vector.memset(
                    t.rearrange("p (r c) -> p r c", r=16)[:, :, HD:VW], 1.0)
                v_sb.append(t)
            ctxT_sb = cpool.tile([128, 2 * S], bf16, tag="ctxT")  # pair p at cols p*S

            with tc.tile_pool(name="ps", bufs=1, space="PSUM") as ps, \
                 tc.tile_pool(name="sb_att", bufs=8) as sb_att, \
                 tc.tile_pool(name="sb_p", bufs=34) as sb_p, \
                 tc.tile_pool(name="sb_out", bufs=4) as sb_out:

                # ---- deferred-PE-work queues: `fill` drains between
                # scores; `fill_late` holds deferrable work (early rows'
                # output projections) reserved for the ACT-bound tail ----
                fill = deque()
                fill_late = deque()
                aux = deque()     # cheap non-PE emissions (DMAs): drained
                late_ok = [False]  # fully each kb, don't consume PE slots

                def drain(n):
                    while aux:
                        aux.popleft()()
                    for _ in range(n):
                        if fill:
                            fill.popleft()()
                        elif fill_late and late_ok[0]:
                            fill_late.popleft()()
                        else:
                            return

                vo_ping = [0]

                def next_vo():
                    vo_ping[0] ^= 1
                    return ("vo", "vo2")[vo_ping[0]]

                # ---- V projection (as fill closures) ----
                def emit_v(j):
                    # per half-tile i: two sequential row-tile groups in one
                    # bank (one accumulation group at a time), then a copy
                    state = {}

                    def acc(i, rr):
                        def go():
                            _MARK(f"v({j},{i},{rr})")
                            if rr == 0:
                                state[i] = ps.tile([128, 512], f32,
                                                   tag=next_vo(),
                                                   name=f"vacc{j}{i}")
                            vacc = state[i]
                            r = 4 * j + 2 * i + rr
                            for k in range(KC):
                                nc.tensor.matmul(
                                    vacc[:, 256 * rr:256 * rr + 256],
                                    lhsT=xt_half[r // 8][k]
                                    [:, 128 * (r % 8):128 * (r % 8) + 128],
                                    rhs=wv_sb[k],
                                    start=(k == 0), stop=(k == KC - 1))
                        return go

                    def copy(i):
                        def go():
                            vacc = state.pop(i)
                            dst = (v_sb[j].rearrange(
                                "p (rr h c) -> p rr h c", rr=4, h=4)
                                [:, 2 * i:2 * i + 2, :, 0:HD])
                            nc.vector.tensor_copy(
                                dst,
                                vacc.rearrange("p (rr h c) -> p rr h c",
                                               rr=2, h=4))
                        return go

                    for i in range(2):
                        fill.append(acc(i, 0))
                        fill.append(acc(i, 1))
                        fill.append(copy(i))

                # ---- Phase 1a: QT/KT for the first seq half (q,k < 1024),
                # k-outer across all 8 accumulators (all 8 PSUM banks) so
                # the k=0 matmuls run on just the c0 slivers + first chunk
                # while the remaining weight/x DMAs stream in ----
                emit_v(0)
                xh = xt_half[0]
                for m in range(2):
                    kacc = ps.tile([128, 1024], f32, tag=f"sc{m}",
                                   name=f"kacc0{m}")
                    qacc = [ps.tile([128, 512], f32, tag=("ctxA", "ctxB")[i],
                                    name=f"qa{m}{i}")
                            for i in range(2)]
                    for k in range(KC):
                        for i in range(2):
                            nc.tensor.matmul(
                                kacc[:, 512 * i:512 * i + 512],
                                lhsT=wk_sb[k][:, 128 * m:128 * m + 128],
                                rhs=xh[k][:, 512 * i:512 * i + 512],
                                start=(k == 0), stop=(k == KC - 1))
                        for i in range(2):
                            nc.tensor.matmul(
                                qacc[i],
                                lhsT=wq_sb[k][:, 128 * m:128 * m + 128],
                                rhs=xh[k][:, 512 * i:512 * i + 512],
                                start=(k == 0), stop=(k == KC - 1))
                        if m == 1:
                            drain(1)  # V(0) fills the m1 pass
                    # all copies on ACT (idle during phase 1), n0 halves
                    # first so the next consumer unblocks sooner; m0 copies
                    # overlap the m1 matmul pass
                    for i in range(2):
                        nc.scalar.copy(
                            kT_sb[:, m * S + 512 * i:m * S + 512 * i + 512],
                            kacc[:, 512 * i:512 * i + 512])
                        nc.scalar.copy(
                            qT_sb[:, m * S + 512 * i:m * S + 512 * i + 512],
                            qacc[i])

                # ---- Phase 1b: second seq half (q,k >= 1024), enqueued as
                # PE fill into attention j0/j1 (which only touch half 0).
                # Accumulators ride the vo/vo2 ring as [128,512] groups. ----
                p1b_left = [0]

                def emit_p1b(m):
                    state = {}

                    def acc(which, i):
                        # which: 'k' or 'q'; i: 512-col n-block within half 1
                        def go():
                            _MARK(f"p1b({m})")
                            p1b_left[0] -= 1
                            t = ps.tile([128, 512], f32, tag=next_vo(),
                                        name=f"p1b{m}{which}{i}")
                            state[(which, i)] = t
                            w_sb = wk_sb if which == "k" else wq_sb
                            for k in range(KC):
                                nc.tensor.matmul(
                                    t,
                                    lhsT=w_sb[k][:, 128 * m:128 * m + 128],
                                    rhs=xt_half[1][k][:, 512 * i:512 * i + 512],
                                    start=(k == 0), stop=(k == KC - 1))
                        return go

                    def copy(which, i):
                        def go():
                            t = state.pop((which, i))
                            dstt = kT_sb if which == "k" else qT_sb
                            base = m * S + 1024 + 512 * i
                            nc.vector.tensor_copy(dstt[:, base:base + 512], t)
                        return go

                    for which in ("k", "q"):
                        for i in range(2):
                            p1b_left[0] += 1
                            fill.append(acc(which, i))
                            fill.append(copy(which, i))

                for m in range(2):
                    emit_p1b(m)

                # ---- Attention helpers ----
                def pv_group(banks, j, pair, p_tiles, qq, hh):
                    # One PSUM accumulation group over kb for (qq, hh); den
                    # rides as rhs col 64. Bank = hh, 65-col region = qq.
                    _MARK(f"pv({j},{pair},{qq},{hh})")
                    if (qq, hh) == (0, 0):
                        banks[0] = ps.tile([128, 4 * VW], f32, tag="ctxA",
                                           name=f"cxa{j}{pair}")
                        banks[1] = ps.tile([128, 4 * VW], f32, tag="ctxB",
                                           name=f"cxb{j}{pair}")
                    r = 4 * j + qq
                    ctx = banks[hh][:, VW * qq:VW * qq + VW]
                    for kb in range(r + 1):
                        nc.tensor.matmul(
                            ctx,
                            lhsT=p_tiles[kb][:, 512 * hh + 128 * qq:
                                             512 * hh + 128 * qq + 128],
                            rhs=v_sb[kb // 4][:, ((kb % 4) * 4 + 2 * pair + hh) * VW:
                                              ((kb % 4) * 4 + 2 * pair + hh + 1) * VW],
                            start=(kb == 0), stop=(kb == r))

                def pv_finish(banks, j, pair, qq):
                    # recip + normalize + transpose [128q,128hd] -> ctxT;
                    # for pair 1 the row's output projection follows, queued
                    # AFTER the transpose so emission order matches deps
                    r = 4 * j + qq
                    rb = sb_att.tile([128, 2], f32, tag="rb", name="rb")
                    cn = sb_att.tile([128, 128], bf16, tag="cn", name="cn")
                    for hh in range(2):
                        ctx = banks[hh][:, VW * qq:VW * qq + VW]
                        nc.vector.reciprocal(rb[:, hh:hh + 1], ctx[:, HD:VW])
                        nc.vector.tensor_scalar_mul(
                            cn[:, 64 * hh:64 * hh + 64],
                            ctx[:, 0:HD], rb[:, hh:hh + 1])
                    dst = ctxT_sb[:, pair * S + 128 * r:pair * S + 128 * r + 128]
                    if USE_DMA_TRANSPOSE and not (j == NQ - 1 and pair == 1):
                        # deferred via the fill queue: SP.SEQ then dispatches
                        # it after the norm has executed, so it doesn't sit
                        # on SP.SEQ waiting (which delayed out-DMAs ~2.4us)
                        aux.append(lambda dst=dst, cn=cn:
                                   nc.sync.dma_start_transpose(out=dst, in_=cn))
                    else:
                        tp = ps.tile([128, 128], bf16, tag=next_vo(), name="tp")
                        nc.tensor.transpose(tp, cn, ident_sb)
                        nc.vector.tensor_copy(dst, tp)
                    if pair == 1:
                        enqueue_outproj(r)

                def enqueue_outproj(mr):
                    # rows of j0/j1 defer to the ACT-bound j3 stretch; they
                    # get their own o_t tag so the late-drained DMA readers
                    # can't interleave with the normal ring's slot reuse
                    late = mr < 12
                    q = fill_late if late else fill
                    o_t = sb_out.tile([128, 1024], bf16,
                                      tag="otL" if late else "ot",
                                      bufs=8 if late else None,
                                      name=f"ot{mr}")

                    def mk(nn):
                        def go():
                            _MARK(f"op({mr},{nn})")
                            oacc = ps.tile([128, 512], f32, name="oacc",
                                           tag=next_vo())
                            for p in range(2):
                                nc.tensor.matmul(
                                    oacc,
                                    lhsT=ctxT_sb[:, p * S + 128 * mr:
                                                 p * S + 128 * mr + 128],
                                    rhs=wo_sb[:, p * D + 512 * nn:
                                              p * D + 512 * nn + 512],
                                    start=(p == 0), stop=(p == 1))
                            if mr == S // 128 - 1:
                                # final row: ACT is idle after the last exp,
                                # and per-half DMAs shorten the tail chain
                                nc.scalar.copy(
                                    o_t[:, 512 * nn:512 * nn + 512], oacc)
                                aux.append(lambda nn=nn: nc.sync.dma_start(
                                    out=outp[128 * mr:128 * mr + 128,
                                             512 * nn:512 * nn + 512],
                                    in_=o_t[:, 512 * nn:512 * nn + 512]))
                            else:
                                nc.vector.tensor_copy(
                                    o_t[:, 512 * nn:512 * nn + 512], oacc)
                                if nn == 1:
                                    # deferred: SP.SEQ must not block on it
                                    aux.append(lambda: nc.sync.dma_start(
                                        out=outp[128 * mr:128 * mr + 128, :],
                                        in_=o_t))
                        return go

                    for nn in range(2):
                        q.append(mk(nn))

                # ---- Attention blocks ----
                for j in range(NQ):
                    if j == 2:
                        _MARK("j2-flush")
                        # scores from here need the half-1 qT/kT: drain just
                        # until the phase-1b closures have been emitted
                        while p1b_left[0] > 0:
                            drain(1)
                    nkb = 4 * (j + 1)
                    for pair in range(2):
                        _MARK(f"A({j},{pair})")
                        if (j, pair) >= (2, 0):
                            late_ok[0] = True
                        if (j, pair) == (NQ - 1, 1):
                            # clear all deferred work before the last block
                            # so the post-last-exp tail stays short
                            while fill or fill_late:
                                drain(1)
                        if pair == 0 and j + 1 < NQ:
                            emit_v(j + 1)
                        p_tiles = []
                        banks = {}
                        for kb in range(nkb):
                            _MARK(f"A({j},{pair})k{kb}")
                            vv = kb - 4 * j  # >= 0 on diagonal tiles
                            off = 128 * vv if vv > 0 else 0
                            sc = ps.tile([128, 1024], f32,
                                         tag="sc%d" % (kb % 2), name="sc")
                            for hh in range(2):
                                nc.tensor.matmul(
                                    sc[:, 512 * hh + off:512 * hh + 512],
                                    lhsT=kT_sb[64 * hh:64 * hh + 64,
                                               pair * S + 128 * kb:
                                               pair * S + 128 * kb + 128],
                                    rhs=qT_sb[64 * hh:64 * hh + 64,
                                              pair * S + 512 * j + off:
                                              pair * S + 512 * j + 512],
                                    start=True, stop=True)
                            p_t = sb_p.tile([128, 1024], bf16, tag="p")
                            if off:
                                sc3 = sc.rearrange("p (h w) -> p h w", h=2)[:, :, off:]
                                pt3 = p_t.rearrange("p (h w) -> p h w", h=2)[:, :, off:]
                                nc.scalar.activation(pt3, sc3, AF.Exp, scale=0.125)
                            else:
                                nc.scalar.activation(p_t, sc, AF.Exp, scale=0.125)
                            if vv >= 0:
                                # triangular mask only on the diag 128-block
                                m3 = (mask_sb[:, 0:256]
                                      .rearrange("p (h w) -> p h w", h=2))
                                pt3 = (p_t.rearrange("p (h w) -> p h w", h=2)
                                       [:, :, off:off + 128])
                                nc.gpsimd.tensor_mul(pt3, pt3, m3)
                            p_tiles.append(p_t)
                            if vv >= 0:
                                # this kb completes q-subtile vv. Last block:
                                # run PV inline (nothing left to overlap).
                                # Other blocks: enqueue, so the PV spreads
                                # into the next block's ACT-bound stretches.
                                last = (j, pair) == (NQ - 1, 1)
                                if last:
                                    if vv == 0:
                                        # the queued PV of (3,0) must fully
                                        # emit before these inline groups
                                        # reuse the ctx banks
                                        while fill:
                                            drain(1)
                                    for hh in range(2):
                                        pv_group(banks, j, pair, p_tiles, vv, hh)
                                    pv_finish(banks, j, pair, vv)
                                else:
                                    def pv_clo(b=banks, jj=j, pp=pair,
                                               pt=p_tiles, qq=vv, hh=0):
                                        def go():
                                            pv_group(b, jj, pp, pt, qq, hh)
                                            if hh == 1:
                                                pv_finish(b, jj, pp, qq)
                                        return go
                                    fill.append(pv_clo(hh=0))
                                    fill.append(pv_clo(hh=1))
                            drain(2 + (len(fill) > 12) + (len(fill) > 20))
                _MARK("final-drain")
                late_ok[0] = True
                while fill or fill_late or aux:
                    drain(1)
                _MARK("end")
    nc.compile()
    return nc


def _make_masks():
    # cols [0:128] and [128:256]: keep(q >= k) triangle for the diagonal
    # 128x128 block (adjacent so one [2,128] AP covers both packed heads);
    # cols [1024:1152]: identity (PE-transpose fallback).
    kr = np.arange(128)[:, None]
    qc = np.arange(128)[None, :]
    m = np.zeros((128, 1152), BF16)
    tri = (qc >= kr).astype(BF16)
    m[:, 0:128] = tri
    m[:, 128:256] = tri
    m[:, 1024:1152] = np.eye(128, dtype=BF16)
    return m


def _prepare_in_maps(x, Wq, Wk, Wv, Wo):
    masks = _make_masks()
    in_maps = []
    xT = [np.ascontiguousarray(x[b].T).astype(BF16) for b in range(B)]
    for c in range(N_CORES):
        b, g = c // G, c % G
        sl = slice(GD * g, GD * (g + 1))
        in_maps.append({
            "xT": xT[b],
            "wq": np.ascontiguousarray(Wq[:, sl]).astype(BF16),
            "wk": np.ascontiguousarray(Wk[:, sl]).astype(BF16),
            "wv": np.ascontiguousarray(Wv[:, sl]).astype(BF16),
            "wo": np.ascontiguousarray(Wo[sl, :]).astype(BF16),
            "masks": masks,
        })
    return in_maps


def run_spmd(x, Wq, Wk, Wv, Wo, bo, **spmd_kwargs):
    from concourse.bass_utils import run_bass_kernel_spmd

    if "nc" not in _CACHE:
        _CACHE["nc"] = _build_nc()
    nc = _CACHE["nc"]
    in_maps = _prepare_in_maps(x, Wq, Wk, Wv, Wo)
    res = run_bass_kernel_spmd(nc, in_maps, core_ids=list(range(N_CORES)),
                               **spmd_kwargs)
    out = np.zeros((B, S, D), np.float32)
    for c in range(N_CORES):
        out[c // G] += np.asarray(res.results[c]["out"], np.float32)
    out += np.asarray(bo, np.float32)[None, None, :]
    return out, res


def kernel(x, Wq, Wk, Wv, Wo, bo):
    x = np.asarray(x)
    out, _ = run_spmd(np.asarray(x, np.float32), np.asarray(Wq, np.float32),
                      np.asarray(Wk, np.float32), np.asarray(Wv, np.float32),
                      np.asarray(Wo, np.float32), np.asarray(bo, np.float32))
    return out


# revision 13
# speedup vs baseline: 1.0456x; 1.0456x over previous
"""Causal multi-head attention (B=2, S=2048, D=1024, H=16) on 8 NeuronCores.

Sharding: batch x head-group (2 x 4). Core c handles batch b = c // 4 and
head group g = c % 4 (4 heads = 256 model dims). Each core computes a full
[S, D] partial of the output projection for its batch; the host sums the 4
group partials per batch and adds the bias.

v2 (cost model: matmul time = out free-dim cols x pe_cycle, so the PV matmul
is flipped to put q on the output partitions):
  Phase 1 (per half,m): KT/QT k-outer accumulation, N=512 matmuls.
  Attention per block (j, pair):
    A: for kb ascending: ST = K_h @ Q_h^T [128 k, 512 q] x2 heads;
       P = exp(0.125*ST) (ACT -> SBUF bf16); diag-block mask on DVE.
       All 16 P tiles of the block stay resident.
    B (drained as PE fill into the next block's A): per (qq, hh) one PSUM
       accumulation group over kb: ctx[q, hd|den] += P_kb^T @ [V_kb|1],
       lhsT = P tile [128k,128q], rhs [128k,65] -> N=65, den rides as col 64.
       Then recip (DVE), normalize (tensor_scalar), and a DMA transpose
       [128q,128hd] -> ctxT [128hd,128q].
  out_partial = ctxT^T @ Wo_g [2048, 1024] -> bf16 -> DRAM.
PSUM banks: sc0(2) sc1(2) ctxA ctxB vo vo2.
"""

from collections import deque

import numpy as np
import ml_dtypes

B, S, D, H = 2, 2048, 1024, 16
HD = 64
G = 4            # head groups = tensor-parallel degree per batch
GD = D // G      # 256 dims per group
N_CORES = 8
BF16 = ml_dtypes.bfloat16

USE_DMA_TRANSPOSE = True

_CACHE = {}


def _MARK(label):  # analysis hook, no-op in production
    pass


def _build_nc():
    import concourse.mybir as mybir
    from concourse import bacc
    from concourse.tile import TileContext

    dt = mybir.dt
    f32 = dt.float32
    bf16 = dt.bfloat16
    AF = mybir.ActivationFunctionType

    nc = bacc.Bacc("TRN2")

    xT = nc.dram_tensor("xT", [D, S], bf16, kind="ExternalInput")
    wq = nc.dram_tensor("wq", [D, GD], bf16, kind="ExternalInput")
    wk = nc.dram_tensor("wk", [D, GD], bf16, kind="ExternalInput")
    wv = nc.dram_tensor("wv", [D, GD], bf16, kind="ExternalInput")
    wo = nc.dram_tensor("wo", [GD, D], bf16, kind="ExternalInput")
    masks = nc.dram_tensor("masks", [128, 1152], bf16, kind="ExternalInput")
    outp = nc.dram_tensor("out", [S, D], bf16, kind="ExternalOutput")

    KC = D // 128   # 8 contraction chunks of 128
    NQ = S // 512   # 4 q blocks
    VW = HD + 1     # 65: V columns + the denominator ones-column

    with TileContext(nc) as tc:
        with tc.tile_pool(name="const", bufs=1) as cpool:
            HS = S // 2

            def load_xt(k, hf):
                t = cpool.tile([128, HS], bf16, tag=f"xT{k}_{hf}",
                               name=f"xt{k}_{hf}")
                nc.sync.dma_start(
                    out=t, in_=xT[128 * k:128 * (k + 1), HS * hf:HS * (hf + 1)])
                return t

            def load_w(nm, dram):
                t = cpool.tile([128, KC * GD], bf16, tag=nm, name=nm + "_t")
                nc.sync.dma_start(
                    out=t.rearrange("p (c g) -> p c g", c=KC),
                    in_=dram.rearrange("(c p) g -> p c g", p=128))
                return [t[:, k * GD:(k + 1) * GD] for k in range(KC)]

            # c0 weight slivers + first xT half-chunk first so PE starts
            # ASAP; the first chunk is split so the k=0 matmul can start
            # after only 512 columns have landed
            wkc0 = cpool.tile([128, GD], bf16, tag="wkc0")
            nc.sync.dma_start(out=wkc0, in_=wk[0:128, :])
            xt00 = cpool.tile([128, HS], bf16, tag="xT0_0", name="xt0_0")
            nc.sync.dma_start(out=xt00[:, 0:512], in_=xT[0:128, 0:512])
            nc.sync.dma_start(out=xt00[:, 512:HS], in_=xT[0:128, 512:HS])
            wqc0 = cpool.tile([128, GD], bf16, tag="wqc0")
            nc.sync.dma_start(out=wqc0, in_=wq[0:128, :])
            xt_a = [xt00]
            wk_rest = cpool.tile([128, (KC - 1) * GD], bf16, tag="wk")
            nc.sync.dma_start(
                out=wk_rest[:, 0:2 * GD].rearrange("p (c g) -> p c g", c=2),
                in_=wk[128:384, :].rearrange("(c p) g -> p c g", p=128))
            xt_a.append(load_xt(1, 0))
            nc.sync.dma_start(
                out=wk_rest[:, 2 * GD:].rearrange("p (c g) -> p c g", c=KC - 3),
                in_=wk[384:, :].rearrange("(c p) g -> p c g", p=128))
            wq_rest = cpool.tile([128, (KC - 1) * GD], bf16, tag="wq")
            nc.sync.dma_start(
                out=wq_rest[:, 0:2 * GD].rearrange("p (c g) -> p c g", c=2),
                in_=wq[128:384, :].rearrange("(c p) g -> p c g", p=128))
            nc.sync.dma_start(
                out=wq_rest[:, 2 * GD:].rearrange("p (c g) -> p c g", c=KC - 3),
                in_=wq[384:, :].rearrange("(c p) g -> p c g", p=128))
            wq_sb = [wqc0] + [wq_rest[:, k * GD:(k + 1) * GD] for k in range(KC - 1)]
            wk_sb = [wkc0] + [wk_rest[:, k * GD:(k + 1) * GD] for k in range(KC - 1)]
            for k in range(2, KC):
                xt_a.append(load_xt(k, 0))
            wv_sb = load_w("wv", wv)
            mask_sb = cpool.tile([128, 1152], bf16, tag="masks")
            nc.sync.dma_start(out=mask_sb, in_=masks[:, :])
            xt_b = [load_xt(k, 1) for k in range(KC)]
            wo_sb = cpool.tile([128, 2 * D], bf16, tag="wo")
            nc.sync.dma_start(
                out=wo_sb.rearrange("p (c o) -> p c o", c=2),
                in_=wo.rearrange("(c p) o -> p c o", p=128))
            xt_half = (xt_a, xt_b)
            ident_sb = mask_sb[:, 1024:1152]

            # persistent per-core intermediates
            qT_sb = cpool.tile([128, 2 * S], bf16, tag="qT")    # pair m at cols m*S
            kT_sb = cpool.tile([128, 2 * S], bf16, tag="kT")
            # V row tile r at cols (r%4)*4*VW, head h at +h*VW; col 64 = ones
            v_sb = []
            for jj in range(NQ):
                t = cpool.tile([128, 4 * 4 * VW], bf16, tag=f"v{jj}",
                               name=f"v{jj}")
                nc.vector.memset(
                    t.rearrange("p (r c) -> p r c", r=16)[:, :, HD:VW], 1.0)
                v_sb.append(t)
            ctxT_sb = cpool.tile([128, 2 * S], bf16, tag="ctxT")  # pair p at cols p*S

            with tc.tile_pool(name="ps", bufs=1, space="PSUM") as ps, \
                 tc.tile_pool(name="sb_att", bufs=8) as sb_att, \
                 tc.tile_pool(name="sb_p", bufs=34) as sb_p, \
                 tc.tile_pool(name="sb_out", bufs=4) as sb_out:

                # ---- deferred-PE-work queues: `fill` drains between
                # scores; `fill_late` holds deferrable work (early rows'
                # output projections) reserved for the ACT-bound tail ----
                fill = deque()
                fill_late = deque()
                aux = deque()     # cheap non-PE emissions (DMAs): drained
                late_ok = [False]  # fully each kb, don't consume PE slots

                def drain(n):
                    while aux:
                        aux.popleft()()
                    for _ in range(n):
                        if fill:
                            fill.popleft()()
                        elif fill_late and late_ok[0]:
                            fill_late.popleft()()
                        else:
                            return

                vo_ping = [0]

                def next_vo():
                    vo_ping[0] ^= 1
                    return ("vo", "vo2")[vo_ping[0]]

                # ---- V projection (as fill closures) ----
                def emit_v(j):
                    # per half-tile i: two sequential row-tile groups in one
                    # bank (one accumulation group at a time), then a copy
                    state = {}

                    def acc(i, rr):
                        def go():
                            _MARK(f"v({j},{i},{rr})")
                            if rr == 0:
                                state[i] = ps.tile([128, 512], f32,
                                                   tag=next_vo(),
                                                   name=f"vacc{j}{i}")
                            vacc = state[i]
                            r = 4 * j + 2 * i + rr
                            for k in range(KC):
                                nc.tensor.matmul(
                                    vacc[:, 256 * rr:256 * rr + 256],
                                    lhsT=xt_half[r // 8][k]
                                    [:, 128 * (r % 8):128 * (r % 8) + 128],
                                    rhs=wv_sb[k],
                                    start=(k == 0), stop=(k == KC - 1))
                        return go

                    def copy(i):
                        def go():
                            vacc = state.pop(i)
                            dst = (v_sb[j].rearrange(
                                "p (rr h c) -> p rr h c", rr=4, h=4)
                                [:, 2 * i:2 * i + 2, :, 0:HD])
                            nc.vector.tensor_copy(
                                dst,
                                vacc.rearrange("p (rr h c) -> p rr h c",
                                               rr=2, h=4))
                        return go

                    for i in range(2):
                        fill.append(acc(i, 0))
                        fill.append(acc(i, 1))
                        fill.append(copy(i))

                # ---- Phase 1a: QT/KT for the first seq half (q,k < 1024),
                # k-outer across all 8 accumulators (all 8 PSUM banks) so
                # the k=0 matmuls run on just the c0 slivers + first chunk
                # while the remaining weight/x DMAs stream in ----
                emit_v(0)
                xh = xt_half[0]
                for m in range(2):
                    kacc = ps.tile([128, 1024], f32, tag=f"sc{m}",
                                   name=f"kacc0{m}")
                    qacc = [ps.tile([128, 512], f32, tag=("ctxA", "ctxB")[i],
                                    name=f"qa{m}{i}")
                            for i in range(2)]
                    for k in range(KC):
                        for i in range(2):
                            nc.tensor.matmul(
                                kacc[:, 512 * i:512 * i + 512],
                                lhsT=wk_sb[k][:, 128 * m:128 * m + 128],
                                rhs=xh[k][:, 512 * i:512 * i + 512],
                                start=(k == 0), stop=(k == KC - 1))
                        for i in range(2):
                            nc.tensor.matmul(
                                qacc[i],
                                lhsT=wq_sb[k][:, 128 * m:128 * m + 128],
                                rhs=xh[k][:, 512 * i:512 * i + 512],
                                start=(k == 0), stop=(k == KC - 1))
                        if m == 1:
                            drain(1)  # V(0) fills the m1 pass
                    # all copies on ACT (idle during phase 1), n0 halves
                    # first so the next consumer unblocks sooner; m0 copies
                    # overlap the m1 matmul pass
                    for i in range(2):
                        nc.scalar.copy(
                            kT_sb[:, m * S + 512 * i:m * S + 512 * i + 512],
                            kacc[:, 512 * i:512 * i + 512])
                        nc.scalar.copy(
                            qT_sb[:, m * S + 512 * i:m * S + 512 * i + 512],
                            qacc[i])

                # ---- Phase 1b: second seq half (q,k >= 1024), enqueued as
                # PE fill into attention j0/j1 (which only touch half 0).
                # Accumulators ride the vo/vo2 ring as [128,512] groups. ----
                p1b_left = [0]

                def emit_p1b(m):
                    state = {}

                    def acc(which, i):
                        # which: 'k' or 'q'; i: 512-col n-block within half 1
                        def go():
                            _MARK(f"p1b({m})")
                            p1b_left[0] -= 1
                            t = ps.tile([128, 512], f32, tag=next_vo(),
                                        name=f"p1b{m}{which}{i}")
                            state[(which, i)] = t
                            w_sb = wk_sb if which == "k" else wq_sb
                            for k in range(KC):
                                nc.tensor.matmul(
                                    t,
                                    lhsT=w_sb[k][:, 128 * m:128 * m + 128],
                                    rhs=xt_half[1][k][:, 512 * i:512 * i + 512],
                                    start=(k == 0), stop=(k == KC - 1))
                        return go

                    def copy(which, i):
                        def go():
                            t = state.pop((which, i))
                            dstt = kT_sb if which == "k" else qT_sb
                            base = m * S + 1024 + 512 * i
                            nc.vector.tensor_copy(dstt[:, base:base + 512], t)
                        return go

                    for which in ("k", "q"):
                        for i in range(2):
                            p1b_left[0] += 1
                            fill.append(acc(which, i))
                            fill.append(copy(which, i))

                for m in range(2):
                    emit_p1b(m)

                # ---- Attention helpers ----
                def pv_group(banks, j, pair, p_tiles, qq, hh):
                    # One PSUM accumulation group over kb for (qq, hh); den
                    # rides as rhs col 64. Bank = hh, 65-col region = qq.
                    _MARK(f"pv({j},{pair},{qq},{hh})")
                    if (qq, hh) == (0, 0):
                        banks[0] = ps.tile([128, 4 * VW], f32, tag="ctxA",
                                           name=f"cxa{j}{pair}")
                        banks[1] = ps.tile([128, 4 * VW], f32, tag="ctxB",
                                           name=f"cxb{j}{pair}")
                    r = 4 * j + qq
                    ctx = banks[hh][:, VW * qq:VW * qq + VW]
                    for kb in range(r + 1):
                        nc.tensor.matmul(
                            ctx,
                            lhsT=p_tiles[kb][:, 512 * hh + 128 * qq:
                                             512 * hh + 128 * qq + 128],
                            rhs=v_sb[kb // 4][:, ((kb % 4) * 4 + 2 * pair + hh) * VW:
                                              ((kb % 4) * 4 + 2 * pair + hh + 1) * VW],
                            start=(kb == 0), stop=(kb == r))

                def pv_finish(banks, j, pair, qq):
                    # recip + normalize + transpose [128q,128hd] -> ctxT;
                    # for pair 1 the row's output projection follows, queued
                    # AFTER the transpose so emission order matches deps
                    r = 4 * j + qq
                    rb = sb_att.tile([128, 2], f32, tag="rb", name="rb")
                    cn = sb_att.tile([128, 128], bf16, tag="cn", name="cn")
                    for hh in range(2):
                        ctx = banks[hh][:, VW * qq:VW * qq + VW]
                        nc.vector.reciprocal(rb[:, hh:hh + 1], ctx[:, HD:VW])
                        nc.vector.tensor_scalar_mul(
                            cn[:, 64 * hh:64 * hh + 64],
                            ctx[:, 0:HD], rb[:, hh:hh + 1])
                    dst = ctxT_sb[:, pair * S + 128 * r:pair * S + 128 * r + 128]
                    if USE_DMA_TRANSPOSE and not (j == NQ - 1 and pair == 1):
                        # deferred via the fill queue: SP.SEQ then dispatches
                        # it after the norm has executed, so it doesn't sit
                        # on SP.SEQ waiting (which delayed out-DMAs ~2.4us)
                        aux.append(lambda dst=dst, cn=cn:
                                   nc.sync.dma_start_transpose(out=dst, in_=cn))
                    else:
                        tp = ps.tile([128, 128], bf16, tag=next_vo(), name="tp")
                        nc.tensor.transpose(tp, cn, ident_sb)
                        nc.vector.tensor_copy(dst, tp)
                    if pair == 1:
                        enqueue_outproj(r)

                def enqueue_outproj(mr):
                    # rows of j0/j1 defer to the ACT-bound j3 stretch; they
                    # get their own o_t tag so the late-drained DMA readers
                    # can't interleave with the normal ring's slot reuse
                    late = mr < 10
                    q = fill_late if late else fill
                    o_t = sb_out.tile([128, 1024], bf16,
                                      tag="otL" if late else "ot",
                                      bufs=8 if late else None,
                                      name=f"ot{mr}")

                    def mk(nn):
                        def go():
                            _MARK(f"op({mr},{nn})")
                            oacc = ps.tile([128, 512], f32, name="oacc",
                                           tag=next_vo())
                            for p in range(2):
                                nc.tensor.matmul(
                                    oacc,
                                    lhsT=ctxT_sb[:, p * S + 128 * mr:
                                                 p * S + 128 * mr + 128],
                                    rhs=wo_sb[:, p * D + 512 * nn:
                                              p * D + 512 * nn + 512],
                                    start=(p == 0), stop=(p == 1))
                            if mr == S // 128 - 1:
                                # final row: ACT is idle after the last exp;
                                # half-DMAs go out on different queues so
                                # their fixed costs overlap
                                nc.scalar.copy(
                                    o_t[:, 512 * nn:512 * nn + 512], oacc)
                                aux.append(lambda nn=nn: nc.sync.dma_start(
                                    out=outp[128 * mr:128 * mr + 128,
                                             512 * nn:512 * nn + 512],
                                    in_=o_t[:, 512 * nn:512 * nn + 512]))
                            else:
                                nc.vector.tensor_copy(
                                    o_t[:, 512 * nn:512 * nn + 512], oacc)
                                if nn == 1:
                                    # deferred: SP.SEQ must not block on it
                                    aux.append(lambda: nc.sync.dma_start(
                                        out=outp[128 * mr:128 * mr + 128, :],
                                        in_=o_t))
                        return go

                    for nn in range(2):
                        q.append(mk(nn))

                # ---- Attention blocks ----
                for j in range(NQ):
                    if j == 2:
                        _MARK("j2-flush")
                        # scores from here need the half-1 qT/kT: drain just
                        # until the phase-1b closures have been emitted
                        while p1b_left[0] > 0:
                            drain(1)
                    nkb = 4 * (j + 1)
                    for pair in range(2):
                        _MARK(f"A({j},{pair})")
                        if (j, pair) >= (3, 1):
                            late_ok[0] = True
                        if (j, pair) == (NQ - 1, 1):
                            # let remaining deferred work spread across this
                            # block's pre-diagonal kbs (the vv==0 flush is
                            # the ordering backstop)
                            pass
                        if pair == 1 and j + 1 < NQ:
                            emit_v(j + 1)
                        p_tiles = []
                        banks = {}
                        for kb in range(nkb):
                            _MARK(f"A({j},{pair})k{kb}")
                            vv = kb - 4 * j  # >= 0 on diagonal tiles
                            off = 128 * vv if vv > 0 else 0
                            sc = ps.tile([128, 1024], f32,
                                         tag="sc%d" % (kb % 2), name="sc")
                            for hh in range(2):
                                nc.tensor.matmul(
                                    sc[:, 512 * hh + off:512 * hh + 512],
                                    lhsT=kT_sb[64 * hh:64 * hh + 64,
                                               pair * S + 128 * kb:
                                               pair * S + 128 * kb + 128],
                                    rhs=qT_sb[64 * hh:64 * hh + 64,
                                              pair * S + 512 * j + off:
                                              pair * S + 512 * j + 512],
                                    start=True, stop=True)
                            p_t = sb_p.tile([128, 1024], bf16, tag="p")
                            if off:
                                sc3 = sc.rearrange("p (h w) -> p h w", h=2)[:, :, off:]
                                pt3 = p_t.rearrange("p (h w) -> p h w", h=2)[:, :, off:]
                                nc.scalar.activation(pt3, sc3, AF.Exp, scale=0.125)
                            else:
                                nc.scalar.activation(p_t, sc, AF.Exp, scale=0.125)
                            if vv >= 0:
                                # triangular mask only on the diag 128-block
                                m3 = (mask_sb[:, 0:256]
                                      .rearrange("p (h w) -> p h w", h=2))
                                pt3 = (p_t.rearrange("p (h w) -> p h w", h=2)
                                       [:, :, off:off + 128])
                                nc.gpsimd.tensor_mul(pt3, pt3, m3)
                            p_tiles.append(p_t)
                            if vv >= 0:
                                # this kb completes q-subtile vv. Last block:
                                # run PV inline (nothing left to overlap).
                                # Other blocks: enqueue, so the PV spreads
                                # into the next block's ACT-bound stretches.
                                last = (j, pair) == (NQ - 1, 1)
                                if last:
                                    if vv == 0:
                                        # the queued PV of (3,0) must fully
                                        # emit before these inline groups
                                        # reuse the ctx banks
                                        while fill:
                                            drain(1)
                                    for hh in range(2):
                                        pv_group(banks, j, pair, p_tiles, vv, hh)
                                    pv_finish(banks, j, pair, vv)
                                else:
                                    def pv_clo(b=banks, jj=j, pp=pair,
                                               pt=p_tiles, qq=vv, hh=0):
                                        def go():
                                            pv_group(b, jj, pp, pt, qq, hh)
                                            if hh == 1:
                                                pv_finish(b, jj, pp, qq)
                                        return go
                                    fill.append(pv_clo(hh=0))
                                    fill.append(pv_clo(hh=1))
                            drain(2 + (len(fill) > 12) + (len(fill) > 20))
                _MARK("final-drain")
                late_ok[0] = True
                while fill or fill_late or aux:
                    drain(1)
                _MARK("end")
    nc.compile()
    return nc


def _make_masks():
    # cols [0:128] and [128:256]: keep(q >= k) triangle for the diagonal
    # 128x128 block (adjacent so one [2,128] AP covers both packed heads);
    # cols [1024:1152]: identity (PE-transpose fallback).
    kr = np.arange(128)[:, None]
    qc = np.arange(128)[None, :]
    m = np.zeros((128, 1152), BF16)
    tri = (qc >= kr).astype(BF16)
    m[:, 0:128] = tri
    m[:, 128:256] = tri
    m[:, 1024:1152] = np.eye(128, dtype=BF16)
    return m


def _prepare_in_maps(x, Wq, Wk, Wv, Wo):
    masks = _make_masks()
    in_maps = []
    xT = [np.ascontiguousarray(x[b].T).astype(BF16) for b in range(B)]
    for c in range(N_CORES):
        b, g = c // G, c % G
        sl = slice(GD * g, GD * (g + 1))
        in_maps.append({
            "xT": xT[b],
            "wq": np.ascontiguousarray(Wq[:, sl]).astype(BF16),
            "wk": np.ascontiguousarray(Wk[:, sl]).astype(BF16),
            "wv": np.ascontiguousarray(Wv[:, sl]).astype(BF16),
            "wo": np.ascontiguousarray(Wo[sl, :]).astype(BF16),
            "masks": masks,
        })
    return in_maps


def run_spmd(x, Wq, Wk, Wv, Wo, bo, **spmd_kwargs):
    from concourse.bass_utils import run_bass_kernel_spmd

    if "nc" not in _CACHE:
        _CACHE["nc"] = _build_nc()
    nc = _CACHE["nc"]
    in_maps = _prepare_in_maps(x, Wq, Wk, Wv, Wo)
    res = run_bass_kernel_spmd(nc, in_maps, core_ids=list(range(N_CORES)),
                               **spmd_kwargs)
    out = np.zeros((B, S, D), np.float32)
    for c in range(N_CORES):
        out[c // G] += np.asarray(res.results[c]["out"], np.float32)
    out += np.asarray(bo, np.float32)[None, None, :]
    return out, res


def kernel(x, Wq, Wk, Wv, Wo, bo):
    x = np.asarray(x)
    out, _ = run_spmd(np.asarray(x, np.float32), np.asarray(Wq, np.float32),
                      np.asarray(Wk, np.float32), np.asarray(Wv, np.float32),
                      np.asarray(Wo, np.float32), np.asarray(bo, np.float32))
    return out
